# revision 1
# baseline (speedup 1.0000x reference)
"""Trainium2 Bass kernel for CombinedICIRLoss (Kendall tau + ListNet + pairwise margin).

Contract: kernel(predictions, targets) takes FULL [32,1024] f32 inputs, returns the
FULL scalar loss (0-d float32 ndarray). Internally shards batch dim across 8
NeuronCores (4 samples each), runs a Bass/Tile kernel per core, and combines tiny
per-sample partial sums on the host.
"""

import numpy as np

B, N = 32, 1024
NCORES = 8
SPC = B // NCORES          # samples per core
JC = N // 128              # j-chunks per sample
KT_INV = 10.0              # 1 / KT_TEMP
NEG30 = -1.0e30
POI = -1.0e6               # poison for invalid-i entries

_cache = {}


def _patch_tile_drain():
    """This container's walrus build only accepts one semaphore wait per CTRL
    instruction; Tile's final drain attaches one wait per live semaphore.
    Split them across consecutive drains (same engine => sequential => same
    semantics)."""
    from concourse.tile import TileContext
    if getattr(TileContext, "_drainfix", False):
        return
    import bass_rust
    from concourse.vector_clock import ScopedClock

    def patched(self, tick_clock, wait_clock):
        drain_inst = self.nc.sync.drain()
        wait_clock.add_sem_waits(
            drain_inst.ins, ScopedClock({None: tick_clock.global_clock})
        )
        ins = drain_inst.ins
        si = ins.sync_info
        if si is not None and len(si.on_wait) > 1:
            waits = list(si.on_wait)
            ins.sync_info = bass_rust.SyncInfo(
                on_wait=waits[:1], on_update=list(si.on_update)
            )
            for w in waits[1:]:
                d2 = self.nc.sync.drain()
                d2.ins.sync_info = bass_rust.SyncInfo(on_wait=[w], on_update=[])
        self.nc.all_engine_barrier()
        popped = self.nc._tile_sem_poison_stack.pop()
        assert popped is self._sem_poison
        self.nc.clear_and_free_semaphores(list(self.sems.allocated().values()))
        self.nc.all_engine_barrier()

    TileContext._drain_and_barrier = patched
    TileContext._drainfix = True


def _split_multi_waits(nc):
    """This walrus build accepts only one semaphore wait per instruction.
    Hoist extra waits onto single-wait NoOps inserted just before, on the same
    engine (same stream position => identical semantics)."""
    import concourse.mybir as mybir
    import bass_rust

    cnt = 0
    for f in nc.m.functions:
        for bb in f.blocks:
            changed = False
            out = []
            for ins in bb.instructions:
                si = ins.sync_info
                if si is not None and len(si.on_wait) > 1:
                    waits = list(si.on_wait)
                    for w in waits[:-1]:
                        cnt += 1
                        nop = mybir.InstNoOp(
                            name=f"waitfix-{cnt}",
                            engine=ins.engine,
                            sync_info=bass_rust.SyncInfo(on_wait=[w], on_update=[]),
                        )
                        out.append(nop)
                    ins.sync_info = bass_rust.SyncInfo(
                        on_wait=[waits[-1]], on_update=list(si.on_update)
                    )
                    changed = True
                out.append(ins)
            if changed:
                bb.instructions = out
    return cnt


def _build(sign_on_pool=False, q_on_pool=False):
    """Build the per-core Bass module: inputs p,t [4,1024] f32, output
    partials [4,4] f32 = per-sample [conc2, Mv, kl, n_valid]."""
    key = ("nc", sign_on_pool, q_on_pool)
    if key in _cache:
        return _cache[key]
    from contextlib import ExitStack
    import concourse.bass as bass
    import concourse.mybir as mybir
    from concourse.tile import TileContext

    _patch_tile_drain()

    f32 = mybir.dt.float32
    bf16 = mybir.dt.bfloat16
    OP = mybir.AluOpType
    AF = mybir.ActivationFunctionType
    AX = mybir.AxisListType

    nc = bass.Bass("TRN2", target_bir_lowering=False, debug=False)
    p_in = nc.dram_tensor("p", [SPC, N], f32, kind="ExternalInput")
    t_in = nc.dram_tensor("t", [SPC, N], f32, kind="ExternalInput")
    out_d = nc.dram_tensor("partials", [SPC, 4], f32, kind="ExternalOutput")

    with TileContext(nc) as tc, ExitStack() as ctx:
        persist = ctx.enter_context(tc.tile_pool(name="persist", bufs=1))
        bcpool = ctx.enter_context(tc.tile_pool(name="bcpool", bufs=2))
        work = ctx.enter_context(tc.tile_pool(name="work", bufs=4))
        small = ctx.enter_context(tc.tile_pool(name="small", bufs=1))
        psum_k = ctx.enter_context(tc.tile_pool(name="psum_k", bufs=1, space="PSUM"))
        dram = ctx.enter_context(tc.tile_pool(name="dram", bufs=1, space="DRAM"))

        # ---------- setup: flat [4,1024] and partitioned [128,32] views ----------
        p4 = persist.tile([SPC, N], f32, tag="p4")
        t4 = persist.tile([SPC, N], f32, tag="t4")
        nc.sync.dma_start(out=p4[:], in_=p_in[:, :])
        nc.sync.dma_start(out=t4[:], in_=t_in[:, :])

        p_part = persist.tile([128, SPC * JC], f32, tag="p_part")
        t_part = persist.tile([128, SPC * JC], f32, tag="t_part")
        nc.sync.dma_start(out=p_part[:], in_=p_in[:, :].rearrange("s (c k) -> k (s c)", k=128))
        nc.sync.dma_start(out=t_part[:], in_=t_in[:, :].rearrange("s (c k) -> k (s c)", k=128))

        v4 = persist.tile([SPC, N], f32, tag="v4")
        nc.vector.tensor_tensor(v4[:], t4[:], t4[:], OP.is_equal)  # NaN != NaN -> 0
        vm4 = persist.tile([SPC, N], mybir.dt.uint32, tag="vm4")
        nc.vector.tensor_tensor(vm4[:], t4[:], t4[:], OP.is_equal)
        nval = small.tile([SPC, 1], f32, tag="nval")
        nc.vector.reduce_sum(nval[:], v4[:], axis=AX.X)

        negpoi4 = persist.tile([SPC, N], f32, tag="negpoi4")
        nc.gpsimd.memset(negpoi4[:], POI)
        ppoi4 = persist.tile([SPC, N], f32, tag="ppoi4")
        nc.vector.select(ppoi4[:], vm4[:], p4[:], negpoi4[:])
        tpoi4 = persist.tile([SPC, N], f32, tag="tpoi4")
        nc.vector.select(tpoi4[:], vm4[:], t4[:], negpoi4[:])

        # poisoned rows to DRAM scratch; broadcast-with-cast back per sample
        scr_p = dram.tile([SPC, N], f32, tag="scr_p")
        scr_t = dram.tile([SPC, N], f32, tag="scr_t")
        nc.sync.dma_start(out=scr_p[:], in_=ppoi4[:])
        nc.sync.dma_start(out=scr_t[:], in_=tpoi4[:])

        v_part = persist.tile([128, SPC * JC], f32, tag="v_part")
        nc.vector.tensor_tensor(v_part[:], t_part[:], t_part[:], OP.is_equal)
        vm_part = persist.tile([128, SPC * JC], mybir.dt.uint32, tag="vm_part")
        nc.vector.tensor_tensor(vm_part[:], t_part[:], t_part[:], OP.is_equal)
        zeros_part = persist.tile([128, SPC * JC], f32, tag="zeros_part")
        nc.gpsimd.memset(zeros_part[:], 0.0)
        ts_part = persist.tile([128, SPC * JC], f32, tag="ts_part")  # t_safe, j-layout
        nc.vector.select(ts_part[:], vm_part[:], t_part[:], zeros_part[:])
        p10 = persist.tile([128, SPC * JC], f32, tag="p10")
        nc.gpsimd.tensor_scalar(p10[:], p_part[:], KT_INV, None, OP.mult)
        t10 = persist.tile([128, SPC * JC], f32, tag="t10")
        nc.gpsimd.tensor_scalar(t10[:], ts_part[:], KT_INV, None, OP.mult)
        negt = persist.tile([128, SPC * JC], f32, tag="negt")
        nc.gpsimd.tensor_scalar(negt[:], ts_part[:], -1.0, None, OP.mult)
        p_col_bf = persist.tile([128, SPC * JC], bf16, tag="p_col_bf")
        nc.gpsimd.tensor_copy(p_col_bf[:], p_part[:])
        t_col_bf = persist.tile([128, SPC * JC], bf16, tag="t_col_bf")
        nc.gpsimd.tensor_copy(t_col_bf[:], ts_part[:])

        # mask-selector stationary (bf16) for the K reduction: for tile c
        # (sample s), cols [4c..4c+4) are zero except col 4c+s = v_part[:, c]
        vsel = persist.tile([128, 4 * SPC * JC], bf16, tag="vsel")
        nc.gpsimd.memset(vsel[:], 0.0)
        for c in range(SPC * JC):
            s = c // JC
            nc.gpsimd.tensor_copy(vsel[:, 4 * c + s : 4 * c + s + 1], v_part[:, c : c + 1])

        ones_col = persist.tile([128, 1], f32, tag="ones_col")
        nc.vector.memset(ones_col[:], 1.0)

        mincol = persist.tile([128, SPC * JC], f32, tag="mincol")
        nc.gpsimd.memset(mincol[:], 0.0)

        K4 = psum_k.tile([SPC, N], f32, tag="K4")
        K4d = psum_k.tile([SPC, N], f32, tag="K4d")

        mincol_d = persist.tile([128, SPC * JC], f32, tag="mincol_d")
        nc.gpsimd.memset(mincol_d[:], 0.0)

        # ---------- main O(N^2/2) loop (upper-triangular chunks) ----------
        # z and min(q,1) are symmetric in (i,j): compute only i >= jc*128.
        # All-ordered sum = 2*S - D where D is the diagonal 128-block part.
        for s in range(SPC):
            # broadcast poisoned rows across 128 partitions, f32 -> bf16, via DMA
            pb = bcpool.tile([128, N], bf16, tag="pb")
            tb = bcpool.tile([128, N], bf16, tag="tb")
            rp = scr_p[s : s + 1, :]
            nc.gpsimd.dma_start(out=pb[:], in_=bass.AP(
                tensor=rp.tensor, offset=rp.offset, ap=[[0, 128]] + list(rp.ap[1:])))
            rt = scr_t[s : s + 1, :]
            nc.gpsimd.dma_start(out=tb[:], in_=bass.AP(
                tensor=rt.tensor, offset=rt.offset, ap=[[0, 128]] + list(rt.ap[1:])))
            for jc in range(JC):
                c = s * JC + jc
                i0 = jc * 128
                L = N - i0
                ps_t = work.tile([128, N], bf16, tag="ps")
                nc.scalar.activation(ps_t[:, :L], pb[:, i0:], AF.Tanh,
                                     bias=p10[:, c : c + 1], scale=-KT_INV)
                ts_t = work.tile([128, N], bf16, tag="ts")
                nc.scalar.activation(ts_t[:, :L], tb[:, i0:], AF.Tanh,
                                     bias=t10[:, c : c + 1], scale=-KT_INV)
                z_t = work.tile([128, N], bf16, tag="z")
                nc.vector.tensor_tensor(z_t[:, :L], ps_t[:, :L], ts_t[:, :L], OP.mult)
                # K4[:, g] += vsel.T @ z over 512-aligned global column chunks
                b0 = i0 // 512
                for bidx in range(b0, 2):
                    g0, g1 = max(i0, bidx * 512), (bidx + 1) * 512
                    nc.tensor.matmul(K4[:, g0:g1], vsel[:, 4 * c : 4 * c + 4],
                                     z_t[:, g0 - i0 : g1 - i0],
                                     start=(s == 0 and jc == 0),
                                     stop=(s == SPC - 1 and jc == JC - 1 and bidx == 1),
                                     skip_group_check=True)
                # diagonal 128-block, accumulated across samples per jc
                nc.tensor.matmul(K4d[:, i0 : i0 + 128], vsel[:, 4 * c : 4 * c + 4],
                                 z_t[:, 0:128], start=(s == 0), stop=(s == SPC - 1),
                                 skip_group_check=True)
                if sign_on_pool:
                    g_t = work.tile([128, N], bf16, tag="g")
                    nc.gpsimd.tensor_scalar(g_t[:, :L], tb[:, i0:],
                                            ts_part[:, c : c + 1], 0.0,
                                            OP.subtract, OP.is_gt)
                    s_t = work.tile([128, N], bf16, tag="sg")
                    nc.gpsimd.tensor_scalar(s_t[:, :L], g_t[:, :L], 2.0, -1.0,
                                            OP.mult, OP.add)
                else:
                    s_t = work.tile([128, N], bf16, tag="sg")
                    nc.scalar.activation(s_t[:, :L], tb[:, i0:], AF.Sign,
                                         bias=negt[:, c : c + 1], scale=1.0)
                q_t = work.tile([128, N], bf16, tag="q")
                q_eng = nc.gpsimd if q_on_pool else nc.vector
                q_eng.scalar_tensor_tensor(q_t[:, :L], pb[:, i0:],
                                           p_col_bf[:, c : c + 1],
                                           s_t[:, :L], OP.subtract, OP.mult)
                mqd_t = work.tile([128, 128], bf16, tag="mqd")
                nc.vector.tensor_scalar(mqd_t[:], q_t[:, 0:128], 1.0, 0.0,
                                        OP.min, OP.add,
                                        accum_out=mincol_d[:, c : c + 1])
                if L > 128:
                    mq_t = work.tile([128, N], bf16, tag="mq")
                    nc.vector.tensor_scalar(mq_t[:, : L - 128], q_t[:, 128:L], 1.0,
                                            0.0, OP.min, OP.add,
                                            accum_out=mincol[:, c : c + 1])

        # ---------- pairwise-margin tail: Mv[s] = sum_j v_j * mincol_j ----------
        mr4 = persist.tile([128, SPC], f32, tag="mr4")
        junk8 = persist.tile([128, JC], f32, tag="junk8")
        comb = persist.tile([128, SPC * JC], f32, tag="comb")
        # all-ordered sum per j: 2*offdiag + diag
        nc.vector.scalar_tensor_tensor(comb[:], mincol[:], 2.0, mincol_d[:],
                                       OP.mult, OP.add)
        for s in range(SPC):
            nc.vector.tensor_tensor(
                junk8[:], comb[:, s * JC : (s + 1) * JC],
                v_part[:, s * JC : (s + 1) * JC], OP.mult)
            nc.vector.reduce_sum(mr4[:, s : s + 1], junk8[:], axis=AX.X)
        Msum = psum_k.tile([SPC, 1], f32, tag="Msum")
        nc.tensor.matmul(Msum[:], mr4[:, 0:SPC], ones_col[:], start=True, stop=True)

        # ---------- Kendall tail: conc2[s] = sum_i v_i * K4[s,i] ----------
        kv = small.tile([SPC, N], f32, tag="kv")
        nc.vector.tensor_tensor(kv[:], K4[:], v4[:], OP.mult)
        r1 = small.tile([SPC, 1], f32, tag="r1")
        nc.vector.reduce_sum(r1[:], kv[:], axis=AX.X)
        kvd = small.tile([SPC, N], f32, tag="kvd")
        nc.vector.tensor_tensor(kvd[:], K4d[:], v4[:], OP.mult)
        r2 = small.tile([SPC, 1], f32, tag="r2")
        nc.vector.reduce_sum(r2[:], kvd[:], axis=AX.X)
        r1x2 = small.tile([SPC, 1], f32, tag="r1x2")
        nc.vector.tensor_scalar(r1x2[:], r1[:], 2.0, None, OP.mult)
        conc2 = small.tile([SPC, 1], f32, tag="conc2")
        nc.vector.tensor_tensor(conc2[:], r1x2[:], r2[:], OP.subtract)

        # ---------- ListNet ----------
        neg30 = persist.tile([SPC, N], f32, tag="neg30")
        nc.gpsimd.memset(neg30[:], NEG30)
        mp4 = small.tile([SPC, N], f32, tag="mp4")
        nc.vector.select(mp4[:], vm4[:], p4[:], neg30[:])
        mt4 = small.tile([SPC, N], f32, tag="mt4")
        nc.vector.select(mt4[:], vm4[:], t4[:], neg30[:])

        mxp = small.tile([SPC, 1], f32, tag="mxp")
        nc.vector.reduce_max(mxp[:], mp4[:], axis=AX.X)
        nmxp = small.tile([SPC, 1], f32, tag="nmxp")
        nc.vector.tensor_scalar(nmxp[:], mxp[:], -1.0, None, OP.mult)
        mxt = small.tile([SPC, 1], f32, tag="mxt")
        nc.vector.reduce_max(mxt[:], mt4[:], axis=AX.X)
        nmxt = small.tile([SPC, 1], f32, tag="nmxt")
        nc.vector.tensor_scalar(nmxt[:], mxt[:], -1.0, None, OP.mult)

        ep = small.tile([SPC, N], f32, tag="ep")
        sep = small.tile([SPC, 1], f32, tag="sep")
        nc.scalar.activation(ep[:], mp4[:], AF.Exp, bias=nmxp[:], scale=1.0,
                             accum_out=sep[:])
        et = small.tile([SPC, N], f32, tag="et")
        st4 = small.tile([SPC, 1], f32, tag="st4")
        nc.scalar.activation(et[:], mt4[:], AF.Exp, bias=nmxt[:], scale=1.0,
                             accum_out=st4[:])
        lnp = small.tile([SPC, 1], f32, tag="lnp")
        nc.scalar.activation(lnp[:], sep[:], AF.Ln)
        lnt = small.tile([SPC, 1], f32, tag="lnt")
        nc.scalar.activation(lnt[:], st4[:], AF.Ln)

        # sh = (mxp + lnp) - (mxt + lnt)
        sh1 = small.tile([SPC, 1], f32, tag="sh1")
        nc.vector.tensor_tensor(sh1[:], mxp[:], mxt[:], OP.subtract)
        sh2 = small.tile([SPC, 1], f32, tag="sh2")
        nc.vector.tensor_tensor(sh2[:], lnp[:], lnt[:], OP.subtract)
        sh = small.tile([SPC, 1], f32, tag="sh")
        nc.vector.tensor_tensor(sh[:], sh1[:], sh2[:], OP.add)

        d4 = small.tile([SPC, N], f32, tag="d4")
        nc.vector.tensor_tensor(d4[:], mt4[:], mp4[:], OP.subtract)
        w4 = small.tile([SPC, N], f32, tag="w4")
        r4 = small.tile([SPC, 1], f32, tag="r4")
        # w4 = (d4 + sh) * et ; r4 = sum(w4)
        nc.vector.scalar_tensor_tensor(w4[:], d4[:], sh[:], et[:], OP.add, OP.mult,
                                       accum_out=r4[:])
        rst = small.tile([SPC, 1], f32, tag="rst")
        nc.vector.reciprocal(rst[:], st4[:])
        kl4 = small.tile([SPC, 1], f32, tag="kl4")
        nc.vector.tensor_tensor(kl4[:], r4[:], rst[:], OP.mult)

        # ---------- pack + store ----------
        outs = small.tile([SPC, 4], f32, tag="outs")
        nc.vector.tensor_copy(outs[:, 0:1], conc2[:])
        nc.vector.tensor_copy(outs[:, 1:2], Msum[:])
        nc.vector.tensor_copy(outs[:, 2:3], kl4[:])
        nc.vector.tensor_copy(outs[:, 3:4], nval[:])
        nc.sync.dma_start(out=out_d[:, :], in_=outs[:])

    _split_multi_waits(nc)
    _cache[key] = nc
    return nc


def _run_device(predictions, targets):
    from concourse.bass_utils import run_bass_kernel_spmd

    nc = _build()
    p = np.ascontiguousarray(predictions, dtype=np.float32)
    t = np.ascontiguousarray(targets, dtype=np.float32)
    in_maps = [
        {"p": p[c * SPC : (c + 1) * SPC], "t": t[c * SPC : (c + 1) * SPC]}
        for c in range(NCORES)
    ]
    res = run_bass_kernel_spmd(nc, in_maps, core_ids=list(range(NCORES)))
    return np.concatenate([res.results[c]["partials"] for c in range(NCORES)], axis=0)


def _poison_corr(targets):
    """Exact correction for the asymmetric poison (invalid-i) contribution in
    the triangular 2S-D reconstruction of Mv, from the NaN mask alone.

    Device Mv counts each (valid j, invalid i) pair's min=1 contribution
    2x if chunk(i) > chunk(j), 1x if same chunk, 0x if below; the true
    all-ordered count is 1x each. corr = sum_j v_j*(2*above_j + own_j)
    - n*(1024-n)."""
    v = ~np.isnan(np.asarray(targets))
    corr = np.zeros(v.shape[0])
    for s in range(v.shape[0]):
        inv = (~v[s]).reshape(JC * NCORES // NCORES, -1) if False else (~v[s]).reshape(-1, 128)
        inv_per_chunk = inv.sum(axis=1).astype(np.float64)      # [8]
        n = float(v[s].sum())
        above = np.concatenate([np.cumsum(inv_per_chunk[::-1])[::-1][1:], [0.0]])
        vals_per_chunk = (~(~v[s]).reshape(-1, 128)).sum(axis=1).astype(np.float64)
        corr[s] = float(np.sum(vals_per_chunk * (2.0 * above + inv_per_chunk))) - n * (1024.0 - n)
    return corr


def _combine(partials, corr):
    """partials [B,4] f64-able: cols conc2, Mv_dev, kl, n_valid -> scalar loss."""
    pa = partials.astype(np.float64)
    conc2, Mv, kl, n = pa[:, 0], pa[:, 1] - corr, pa[:, 2], pa[:, 3]
    ok = n > 1
    n_ok = max(int(ok.sum()), 1)
    tri = np.maximum(n * (n - 1) / 2.0, 1.0)
    conc = (conc2 / 2.0) / tri
    pw_num = 1024.0 * n - Mv - n
    pw_den = np.maximum(n * (n - 1), 1.0)
    pw = pw_num / pw_den
    kendall = -np.sum(np.where(ok, conc, 0.0)) / n_ok
    listnet = np.sum(np.where(ok, kl, 0.0)) / n_ok
    pairwise = np.sum(np.where(ok, pw, 0.0)) / n_ok
    return np.float32(kendall + listnet + pairwise)


def kernel(predictions, targets):
    partials = _run_device(predictions, targets)
    return np.asarray(_combine(partials, _poison_corr(targets)), dtype=np.float32)


def estimate_ns():
    """Cost-model (TimelineSim) single-core duration estimate in ns."""
    from concourse.timeline_sim import TimelineSim

    nc = _build()
    sim = TimelineSim(nc)
    return sim.simulate()



# revision 31
# speedup vs baseline: 1.4547x; 1.4547x over previous
"""Trainium2 Bass kernel for CombinedICIRLoss (Kendall tau + ListNet + pairwise margin).

Contract: kernel(predictions, targets) takes FULL [32,1024] f32 inputs, returns the
FULL scalar loss (0-d float32 ndarray). Internally shards batch dim across 8
NeuronCores (4 samples each), runs a Bass/Tile kernel per core, and combines tiny
per-sample partial sums on the host.

v2 engine plan (per core, 4 samples x triangular 1024x1024 pair tiles):
  Act : ts = tanh(-10*tb + 10*t_i)  [per-chunk, bias trick]
        ps = tanh(-10*e)            [fused big instructions over e buffer]
  DVE : e = (pb - p_i)              [tensor_scalar ptr, 4x mode]
        z = ps*ts, pq = e*s2        [tensor_tensor, 2x mode]
        macc = min(-2*pq, 1) accum  [tensor_scalar + accum, 4x mode]
  Pool: s2 = (tb is_lt t_i) - v/2   [scalar_tensor_tensor ptr]
  PE  : K4 += vsel^T @ z            [Kendall masked reduction, triangular 2S-D]
Both v_i and v_j masks are exact on-device (no host poison correction).
"""

import numpy as np

B, N = 32, 1024
NCORES = 8
SPC = B // NCORES          # samples per core
JC = N // 128              # j-chunks per sample
CTOT = SPC * JC            # 32 chunk-columns
TRI = N + (N - 128 * (JC // 2)) * (JC // 2)  # unused; doc only
POI = 1.0e6                # poison for invalid t entries (+large)
NEG30 = -1.0e30
# per-sample concatenated buffer layout: offsets of each jc slice
OFFS = []
_o = 0
for _jc in range(JC):
    OFFS.append(_o)
    _o += N - 128 * _jc
BUFW = _o                  # 4608 columns

_cache = {}
Z_POOL = {1}      # samples whose z-mult runs on Pool
PQ_POOL = {2, 3}  # samples whose pq-mult runs on Pool


def _patch_tile_drain():
    """This container's walrus build only accepts one semaphore wait per CTRL
    instruction; Tile's final drain attaches one wait per live semaphore.
    Split them across consecutive drains (same engine => sequential => same
    semantics)."""
    from concourse.tile import TileContext
    if getattr(TileContext, "_drainfix", False):
        return
    import bass_rust
    from concourse.vector_clock import ScopedClock

    def patched(self, tick_clock, wait_clock):
        drain_inst = self.nc.sync.drain()
        wait_clock.add_sem_waits(
            drain_inst.ins, ScopedClock({None: tick_clock.global_clock})
        )
        ins = drain_inst.ins
        si = ins.sync_info
        if si is not None and len(si.on_wait) > 1:
            waits = list(si.on_wait)
            ins.sync_info = bass_rust.SyncInfo(
                on_wait=waits[:1], on_update=list(si.on_update)
            )
            for w in waits[1:]:
                d2 = self.nc.sync.drain()
                d2.ins.sync_info = bass_rust.SyncInfo(on_wait=[w], on_update=[])
        self.nc.all_engine_barrier()
        popped = self.nc._tile_sem_poison_stack.pop()
        assert popped is self._sem_poison
        self.nc.clear_and_free_semaphores(list(self.sems.allocated().values()))
        self.nc.all_engine_barrier()

    TileContext._drain_and_barrier = patched
    TileContext._drainfix = True


def _split_multi_waits(nc):
    """This walrus build accepts only one semaphore wait per instruction.
    Hoist extra waits onto single-wait NoOps inserted just before, on the same
    engine (same stream position => identical semantics)."""
    import concourse.mybir as mybir
    import bass_rust

    cnt = 0
    for f in nc.m.functions:
        for bb in f.blocks:
            changed = False
            out = []
            for ins in bb.instructions:
                si = ins.sync_info
                if si is not None and len(si.on_wait) > 1:
                    waits = list(si.on_wait)
                    for w in waits[:-1]:
                        cnt += 1
                        nop = mybir.InstNoOp(
                            name=f"waitfix-{cnt}",
                            engine=ins.engine,
                            sync_info=bass_rust.SyncInfo(on_wait=[w], on_update=[]),
                        )
                        out.append(nop)
                    ins.sync_info = bass_rust.SyncInfo(
                        on_wait=[waits[-1]], on_update=list(si.on_update)
                    )
                    changed = True
                out.append(ins)
            if changed:
                bb.instructions = out
    return cnt


def _build():
    """Per-core Bass module: inputs p,t [4,1024] f32, output partials [4,4]
    f32 = per-sample [conc2, Mv, kl, n_valid]."""
    if "nc" in _cache:
        return _cache["nc"]
    from contextlib import ExitStack
    import concourse.bass as bass
    import concourse.mybir as mybir
    from concourse.tile import TileContext

    _patch_tile_drain()

    f32 = mybir.dt.float32
    bf16 = mybir.dt.bfloat16
    u32 = mybir.dt.uint32
    OP = mybir.AluOpType
    AF = mybir.ActivationFunctionType
    AX = mybir.AxisListType

    nc = bass.Bass("TRN2", target_bir_lowering=False, debug=False)
    p_in = nc.dram_tensor("p", [SPC, N], f32, kind="ExternalInput")
    t_in = nc.dram_tensor("t", [SPC, N], f32, kind="ExternalInput")
    out_d = nc.dram_tensor("partials", [SPC, 4], f32, kind="ExternalOutput")

    with TileContext(nc) as tc, ExitStack() as ctx:
        persist = ctx.enter_context(tc.tile_pool(name="persist", bufs=1))
        bcpool = ctx.enter_context(tc.tile_pool(name="bcpool", bufs=2))
        sbuf = ctx.enter_context(tc.tile_pool(name="sbuf", bufs=2))
        small = ctx.enter_context(tc.tile_pool(name="small", bufs=1))
        psum_k = ctx.enter_context(tc.tile_pool(name="psum_k", bufs=1, space="PSUM"))
        dram = ctx.enter_context(tc.tile_pool(name="dram", bufs=1, space="DRAM"))

        # ---------- prologue, ordered for the tb(0) critical path ----------
        # chain: t4 dma -> vm4 -> tpoi4 -> scr_tb cast-dma -> tb(0) -> ts(0)
        t4 = persist.tile([SPC, N], f32, tag="t4")
        nc.sync.dma_start(out=t4[:], in_=t_in[:, :])
        p4 = persist.tile([SPC, N], f32, tag="p4")
        nc.sync.dma_start(out=p4[:], in_=p_in[:, :])

        vm4 = persist.tile([SPC, N], u32, tag="vm4")
        nc.vector.tensor_tensor(vm4[:], t4[:], t4[:], OP.is_equal)
        poi4 = persist.tile([SPC, N], f32, tag="poi4")
        nc.vector.memset(poi4[:], POI)
        tpoi4 = persist.tile([SPC, N], f32, tag="tpoi4")
        nc.vector.select(tpoi4[:], vm4[:], t4[:], poi4[:])
        scr_tb = dram.tile([SPC, N], bf16, tag="scr_tb")
        nc.gpsimd.dma_start(out=scr_tb[:], in_=tpoi4[:])   # cast f32->bf16
        scr_t = dram.tile([SPC, N], f32, tag="scr_t")
        nc.sync.dma_start(out=scr_t[:], in_=tpoi4[:])
        # p poisoned at invalid-t positions -> invalid-j pairwise term == 1
        ppoi4 = persist.tile([SPC, N], f32, tag="ppoi4")
        nc.vector.select(ppoi4[:], vm4[:], p4[:], poi4[:])
        scr_pb = dram.tile([SPC, N], bf16, tag="scr_pb")
        nc.gpsimd.dma_start(out=scr_pb[:], in_=ppoi4[:])   # cast f32->bf16

        # t columns (s2 ptr): gather poisoned t straight from scr_t
        tpoip = persist.tile([128, CTOT], f32, tag="tpoip")
        nc.sync.dma_start(out=tpoip[:], in_=scr_t[:, :].rearrange("s (c k) -> k (s c)", k=128))

        # p columns (e ptr / Act bias): negated for the Act Copy-bias form
        p_part = persist.tile([128, CTOT], f32, tag="p_part")
        nc.scalar.dma_start(out=p_part[:], in_=p_in[:, :].rearrange("s (c k) -> k (s c)", k=128))
        np_part = persist.tile([128, CTOT], f32, tag="np_part")
        nc.gpsimd.tensor_scalar(np_part[:], p_part[:], -1.0, None, OP.mult)

        # accumulator columns for pairwise tail
        mvd = persist.tile([128, CTOT], f32, tag="mvd")   # diag-block sums
        mvo = persist.tile([128, CTOT], f32, tag="mvo")   # off-block sums
        nc.vector.memset(mvd[:], 0.0)
        nc.vector.memset(mvo[:], 0.0)

        K4 = psum_k.tile([SPC, N], f32, tag="K4")
        K4d = psum_k.tile([SPC, N], f32, tag="K4d")

        def late_prologue():
            """Non-critical prep: emitted after sample-0 slices so the
            scheduler keeps the tb(0)/ts(0) path clear."""
            v4 = persist.tile([SPC, N], f32, tag="v4")
            nc.vector.tensor_tensor(v4[:], t4[:], t4[:], OP.is_equal)
            nval = small.tile([SPC, 1], f32, tag="nval")
            nc.vector.reduce_sum(nval[:], v4[:], axis=AX.X)
            # v_part from the poisoned gather: poison (1e6) == itself -> use
            # threshold compare instead of is_equal-NaN trick
            v_part = persist.tile([128, CTOT], f32, tag="v_part")
            nc.vector.tensor_scalar(v_part[:], tpoip[:], 99999.0, None, OP.is_lt)
            # vsel: [128, 4*CTOT] bf16, col 4c+s = v_part[:, c]
            vsel = persist.tile([128, 4 * CTOT], bf16, tag="vsel")
            nc.vector.memset(vsel[:], 0.0)
            for c in range(CTOT):
                s = c // JC
                nc.gpsimd.tensor_copy(vsel[:, 4 * c + s: 4 * c + s + 1], v_part[:, c: c + 1])
            ones_col = persist.tile([128, 1], f32, tag="ones_col")
            nc.vector.memset(ones_col[:], 1.0)
            # ListNet prep
            neg30 = persist.tile([SPC, N], f32, tag="neg30")
            nc.vector.memset(neg30[:], NEG30)
            mp4 = persist.tile([SPC, N], f32, tag="mp4")
            nc.vector.select(mp4[:], vm4[:], p4[:], neg30[:])
            mt4 = persist.tile([SPC, N], f32, tag="mt4")
            nc.vector.select(mt4[:], vm4[:], t4[:], neg30[:])
            mxp = small.tile([SPC, 1], f32, tag="mxp")
            nc.vector.reduce_max(mxp[:], mp4[:], axis=AX.X)
            nmxp = small.tile([SPC, 1], f32, tag="nmxp")
            nc.vector.tensor_scalar(nmxp[:], mxp[:], -1.0, None, OP.mult)
            mxt = small.tile([SPC, 1], f32, tag="mxt")
            nc.vector.reduce_max(mxt[:], mt4[:], axis=AX.X)
            nmxt = small.tile([SPC, 1], f32, tag="nmxt")
            nc.vector.tensor_scalar(nmxt[:], mxt[:], -1.0, None, OP.mult)
            d4 = persist.tile([SPC, N], f32, tag="d4")
            nc.vector.tensor_tensor(d4[:], mt4[:], mp4[:], OP.subtract)
            return v4, nval, v_part, vsel, ones_col, mp4, mt4, mxp, nmxp, mxt, nmxt, d4

        # ---------- main pipelined loop over samples ----------
        HALVES = ((0, 4), (4, 8))  # jc ranges per half

        def bcasts(s):
            pb = bcpool.tile([128, N], bf16, tag="pb")
            rp = scr_pb[s: s + 1, :]
            nc.sync.dma_start(out=pb[:], in_=bass.AP(
                tensor=rp.tensor, offset=rp.offset, ap=[[0, 128]] + list(rp.ap[1:])))
            tb = bcpool.tile([128, N], bf16, tag="tb")
            rt = scr_tb[s: s + 1, :]
            nc.sync.dma_start(out=tb[:], in_=bass.AP(
                tensor=rt.tensor, offset=rt.offset, ap=[[0, 128]] + list(rt.ap[1:])))
            return pb, tb

        # e-chunks computed on Act (Copy-bias) vs DVE (ts-ptr): balance knob
        ACT_E_JC = set()  # Act Identity is table-based (inexact); e must be exact

        def slices(s, pb, tb):
            e_buf = sbuf.tile([128, BUFW], bf16, tag="e_buf")
            s2_buf = sbuf.tile([128, BUFW], bf16, tag="s2_buf")
            for jc in range(JC):
                c = s * JC + jc
                i0 = jc * 128
                L = N - i0
                o = OFFS[jc]
                if jc in ACT_E_JC:
                    nc.scalar.activation(e_buf[:, o:o + L], pb[:, i0:], AF.Identity,
                                         bias=np_part[:, c:c + 1], scale=1.0)
                else:
                    nc.vector.tensor_scalar(e_buf[:, o:o + L], pb[:, i0:],
                                            p_part[:, c:c + 1], None, OP.subtract)
                nc.vector.tensor_scalar(s2_buf[:, o:o + L], tb[:, i0:],
                                        tpoip[:, c:c + 1], 0.5,
                                        OP.is_lt, OP.subtract)
            return e_buf, s2_buf

        def act_ps(s, bufs):
            """Emit the fused tanh over the e buffer (Act queue) right after
            this sample's e slices, so Act never idles between samples."""
            e_buf, s2_buf = bufs
            ps_buf = sbuf.tile([128, BUFW], bf16, tag="ps_buf")
            for (j0, j1) in HALVES:
                o0, o1 = OFFS[j0], (OFFS[j1] if j1 < JC else BUFW)
                nc.scalar.activation(ps_buf[:, o0:o1], e_buf[:, o0:o1], AF.Tanh,
                                     scale=-10.0)
            return ps_buf

        def heavy(s, bufs, ps_buf):
            e_buf, s2_buf = bufs
            z_buf = sbuf.tile([128, BUFW], bf16, tag="z_buf")
            pq_buf = sbuf.tile([128, BUFW], bf16, tag="pq_buf")
            for (j0, j1) in HALVES:
                o0, o1 = OFFS[j0], (OFFS[j1] if j1 < JC else BUFW)
                # z = tanh(10 pd) * sign(td)/2  (half-sign Kendall, v via tail)
                z_eng = nc.gpsimd if s in Z_POOL else nc.vector
                z_eng.tensor_tensor(z_buf[:, o0:o1], ps_buf[:, o0:o1],
                                    s2_buf[:, o0:o1], OP.mult)
                pq_eng = nc.gpsimd if s in PQ_POOL else nc.vector
                pq_eng.tensor_tensor(pq_buf[:, o0:o1], e_buf[:, o0:o1],
                                     s2_buf[:, o0:o1], OP.mult)
                for jc in range(j0, j1):
                    c = s * JC + jc
                    i0 = jc * 128
                    L = N - i0
                    o = OFFS[jc]
                    # Kendall: K4[:, g] += vsel^T @ z over 512-col groups
                    b0 = i0 // 512
                    for bidx in range(b0, 2):
                        g0, g1 = max(i0, bidx * 512), (bidx + 1) * 512
                        nc.tensor.matmul(K4[:, g0:g1], vsel[:, 4 * c: 4 * c + 4],
                                         z_buf[:, o + g0 - i0: o + g1 - i0],
                                         start=(s == 0 and jc == 0),
                                         stop=(s == SPC - 1 and jc == JC - 1 and bidx == 1),
                                         skip_group_check=True)
                    nc.tensor.matmul(K4d[:, i0: i0 + 128], vsel[:, 4 * c: 4 * c + 4],
                                     z_buf[:, o: o + 128], start=(s == 0),
                                     stop=(s == SPC - 1), skip_group_check=True)
                    # pairwise: min(-2*pq, 1) accumulated per i
                    # accum_out sums the STAGE-0 ALU result: put max there;
                    # min(-2*pq,1) == -2*max(pq,-0.5), the -2 applied on host
                    mqd = sbuf.tile([128, 128], bf16, tag="mqd")
                    nc.vector.tensor_scalar(mqd[:], pq_buf[:, o: o + 128], -0.5, 0.0,
                                            OP.max, OP.add, accum_out=mvd[:, c:c + 1])
                    if L > 128:
                        mqo = sbuf.tile([128, N], bf16, tag="mqo")
                        nc.vector.tensor_scalar(mqo[:, : L - 128], pq_buf[:, o + 128: o + L],
                                                -0.5, 0.0, OP.max, OP.add,
                                                accum_out=mvo[:, c:c + 1])

        cur_bc = bcasts(0)
        cur = slices(0, *cur_bc)
        cur_ps = act_ps(0, cur)
        (v4, nval, v_part, vsel, ones_col, mp4, mt4,
         mxp, nmxp, mxt, nmxt, d4) = late_prologue()
        for s in range(SPC):
            if s + 1 < SPC:
                nxt_bc = bcasts(s + 1)
                nxt = slices(s + 1, *nxt_bc)
                nxt_ps = act_ps(s + 1, nxt)
            heavy(s, cur, cur_ps)
            if s + 1 < SPC:
                cur, cur_ps = nxt, nxt_ps

        # ---------- Kendall tail ----------
        # z tiles were ps * (v_j*sign(td))/2, so v_j is already applied and
        # conc2[s] = sum_j (4*K4 - 2*K4d)[s, j].  K4's 512-col groups include
        # the diagonal 128-block (hence -K4d); pairwise mvo excludes it
        # (hence 2*mvo+mvd below).  Act Copy+accum reads PSUM directly.
        ka = small.tile([SPC, N], f32, tag="ka")
        kb = small.tile([SPC, N], f32, tag="kb")
        nc.scalar.activation(ka[:], K4[:], AF.Copy, scale=4.0)
        nc.scalar.activation(kb[:], K4d[:], AF.Copy, scale=-2.0)
        kc = small.tile([SPC, N], f32, tag="kc")
        nc.vector.tensor_tensor(kc[:], ka[:], kb[:], OP.add)
        kjunk = small.tile([SPC, N], f32, tag="kjunk")
        conc2 = small.tile([SPC, 1], f32, tag="conc2")
        nc.vector.scalar_tensor_tensor(kjunk[:], kc[:], 1.0, v4[:], OP.mult,
                                       OP.mult, accum_out=conc2[:])

        # ---------- pairwise tail: Mv[s] = sum_i v_i*(2*mvo + mvd)[i, c in s] -
        comb = small.tile([128, CTOT], f32, tag="comb")
        nc.vector.scalar_tensor_tensor(comb[:], mvo[:], 2.0, mvd[:], OP.mult, OP.add)
        wk = small.tile([128, CTOT], f32, tag="wk")
        nc.vector.tensor_tensor(wk[:], comb[:], v_part[:], OP.mult)
        mr4 = small.tile([128, SPC], f32, tag="mr4")
        for s in range(SPC):
            nc.vector.reduce_sum(mr4[:, s:s + 1], wk[:, s * JC:(s + 1) * JC], axis=AX.X)
        Msum = psum_k.tile([SPC, 1], f32, tag="Msum")
        nc.tensor.matmul(Msum[:], mr4[:, 0:SPC], ones_col[:], start=True, stop=True)

        # ---------- ListNet finish ----------
        ep = small.tile([SPC, N], f32, tag="ep")
        sep = small.tile([SPC, 1], f32, tag="sep")
        nc.scalar.activation(ep[:], mp4[:], AF.Exp, bias=nmxp[:], scale=1.0,
                             accum_out=sep[:])
        et = small.tile([SPC, N], f32, tag="et")
        st4 = small.tile([SPC, 1], f32, tag="st4")
        nc.scalar.activation(et[:], mt4[:], AF.Exp, bias=nmxt[:], scale=1.0,
                             accum_out=st4[:])
        lnp = small.tile([SPC, 1], f32, tag="lnp")
        nc.scalar.activation(lnp[:], sep[:], AF.Ln)
        lnt = small.tile([SPC, 1], f32, tag="lnt")
        nc.scalar.activation(lnt[:], st4[:], AF.Ln)
        sh1 = small.tile([SPC, 1], f32, tag="sh1")
        nc.vector.tensor_tensor(sh1[:], mxp[:], mxt[:], OP.subtract)
        sh2 = small.tile([SPC, 1], f32, tag="sh2")
        nc.vector.tensor_tensor(sh2[:], lnp[:], lnt[:], OP.subtract)
        sh = small.tile([SPC, 1], f32, tag="sh")
        nc.vector.tensor_tensor(sh[:], sh1[:], sh2[:], OP.add)
        w4 = small.tile([SPC, N], f32, tag="w4")
        r4 = small.tile([SPC, 1], f32, tag="r4")
        nc.vector.scalar_tensor_tensor(w4[:], d4[:], sh[:], et[:], OP.add, OP.mult,
                                       accum_out=r4[:])
        rst = small.tile([SPC, 1], f32, tag="rst")
        nc.vector.reciprocal(rst[:], st4[:])
        kl4 = small.tile([SPC, 1], f32, tag="kl4")
        nc.vector.tensor_tensor(kl4[:], r4[:], rst[:], OP.mult)

        # ---------- pack + store ----------
        outs = small.tile([SPC, 4], f32, tag="outs")
        nc.vector.tensor_copy(outs[:, 0:1], conc2[:])
        nc.vector.tensor_copy(outs[:, 1:2], Msum[:])
        nc.vector.tensor_copy(outs[:, 2:3], kl4[:])
        nc.vector.tensor_copy(outs[:, 3:4], nval[:])
        nc.sync.dma_start(out=out_d[:, :], in_=outs[:])

    _split_multi_waits(nc)
    _cache["nc"] = nc
    return nc


def _run_device(predictions, targets):
    from concourse.bass_utils import run_bass_kernel_spmd

    nc = _build()
    p = np.ascontiguousarray(predictions, dtype=np.float32)
    t = np.ascontiguousarray(targets, dtype=np.float32)
    in_maps = [
        {"p": p[c * SPC: (c + 1) * SPC], "t": t[c * SPC: (c + 1) * SPC]}
        for c in range(NCORES)
    ]
    res = run_bass_kernel_spmd(nc, in_maps, core_ids=list(range(NCORES)))
    return np.concatenate([res.results[c]["partials"] for c in range(NCORES)], axis=0)


def _poison_corr(targets):
    """Invalid-j pairwise contributions are exactly 1 on device (p,t poisoned
    to +1e6).  For valid i in chunk k they are counted 2x for chunks>k (mvo)
    and 1x for chunk k (mvd): corr = sum_k vals[k]*(2*above_inv[k]+inv[k])."""
    v = ~np.isnan(np.asarray(targets))
    corr = np.zeros(v.shape[0])
    for s in range(v.shape[0]):
        inv = (~v[s]).reshape(-1, 128).sum(axis=1).astype(np.float64)
        vals = v[s].reshape(-1, 128).sum(axis=1).astype(np.float64)
        above = np.concatenate([np.cumsum(inv[::-1])[::-1][1:], [0.0]])
        corr[s] = float(np.sum(vals * (2.0 * above + inv)))
    return corr


def _combine(partials, corr):
    """partials [B,4]: cols conc2, Mv, kl, n_valid -> scalar loss.
    Mv includes i==j (contributes 0) and the invalid-j constant (corr)."""
    pa = partials.astype(np.float64)
    conc2, Mv, kl, n = pa[:, 0], -2.0 * pa[:, 1] - corr, pa[:, 2], pa[:, 3]
    ok = n > 1
    n_ok = max(int(ok.sum()), 1)
    tri = np.maximum(n * (n - 1) / 2.0, 1.0)
    conc = (conc2 / 2.0) / tri
    pw_num = n * (n - 1) - Mv
    pw_den = np.maximum(n * (n - 1), 1.0)
    pw = pw_num / pw_den
    kendall = -np.sum(np.where(ok, conc, 0.0)) / n_ok
    listnet = np.sum(np.where(ok, kl, 0.0)) / n_ok
    pairwise = np.sum(np.where(ok, pw, 0.0)) / n_ok
    return np.float32(kendall + listnet + pairwise)


def kernel(predictions, targets):
    partials = _run_device(predictions, targets)
    return np.asarray(_combine(partials, _poison_corr(targets)), dtype=np.float32)


def estimate_ns():
    """Cost-model (TimelineSim) single-core duration estimate in ns."""
    from concourse.timeline_sim import TimelineSim

    nc = _build()
    sim = TimelineSim(nc)
    return sim.simulate()
